# revision 2
# baseline (speedup 1.0000x reference)
"""AttentionLSTM v3: single-chain recurrence, fully resident, no in-loop DMA.

Structure per core (BL=8 batch):
  - xte/xtd (input features, bf16, feature-major k-tiles) loaded wholly into
    SBUF; the input projection Wih0 @ x_t is folded into the per-step z0
    accumulation (k-tiles 2,3), so there is no gx prologue, no DRAM spill,
    and no DMA inside the recurrence loop.
  - Layer weights stacked [W_recurrent | W_input] as KT=4 lhsT tiles:
      L0: z0 = b0 + Whh0 @ h0_{t-1} (k=0,1) + Wih0 @ x_t (k=2,3)
      L1: z1 = b1 + Wih1 @ h0_t (k=0,1)    + Whh1 @ h1_{t-1} (k=2,3)
    Issue order per step: bias identity, k=2,3 (operands ready early),
    then k=0,1 (the fresh h) -- the in-order PE stream stalls only on the
    truly recurrent matmuls.
  - Gates: one sigmoid over [i f o 2g] (g rows pre-scaled by 2 on host:
    tanh(g) = 2*sigmoid(2g) - 1), elementwise chain on DVE, tanh on Act.
  - h1 is written by the gate chain directly into the encT/decT attention
    store (contiguous 16-column slice per step) -- no per-step copies.
  - Wavefront: L1 lags L0 by one step.
Attention (scores/softmax/attn_v) unchanged from v1.
"""
import sys

import numpy as np

for _p in ("/opt/trn_rl_repo", "/root/.axon_site/_ro/trn_rl_repo"):
    if _p not in sys.path:
        sys.path.append(_p)

import ml_dtypes  # noqa: E402
import concourse.bass as bass  # noqa: E402
import concourse.bacc as bacc  # noqa: E402
import concourse.mybir as mybir  # noqa: E402
from concourse import tile  # noqa: E402
from concourse.bass_utils import run_bass_kernel_spmd  # noqa: E402

F32 = mybir.dt.float32
BF16 = mybir.dt.bfloat16
AF = mybir.ActivationFunctionType
ALU = mybir.AluOpType
AX = mybir.AxisListType

NCORES = 8
S, T, B = 512, 512, 64
BL = B // NCORES          # 8 batch per core
H = 256
IN = 256
G = 4 * H                 # 1024 gates
KH = H // 128             # 2 k-tiles per 256-dim contraction
KT = 4                    # stacked k-tiles per layer [rec | input]
M8 = G // 128             # 8 gate m-tiles
NB = BL

# torch gate order (i, f, g, o) -> on-chip order (i, f, o, g): one sigmoid
# covers cols [0, 6*NB) and the scaled-g trick covers [6*NB, 8*NB)
GATE_ORDER = np.concatenate(
    [np.arange(0, 2 * H), np.arange(3 * H, 4 * H), np.arange(2 * H, 3 * H)]
)


def build_nc(ns=S, nt=T, reps=1):
    nc = bacc.Bacc("TRN2", target_bir_lowering=False, debug=False,
                   num_devices=NCORES)

    def inp(name, shape, dt):
        return nc.dram_tensor(name, list(shape), dt, kind="ExternalInput")

    xte = inp("xte", (128, KH * ns * BL), BF16)   # col = k*(ns*BL) + t*BL + b
    xtd = inp("xtd", (128, KH * nt * BL), BF16)
    w0e = inp("w0e", (128, KT * M8 * 128), BF16)  # lhsT tiles, col j = m*KT + k
    w1e = inp("w1e", (128, KT * M8 * 128), BF16)
    w0d = inp("w0d", (128, KT * M8 * 128), BF16)
    w1d = inp("w1d", (128, KT * M8 * 128), BF16)
    b0e = inp("b0e", (128, M8 * BL), BF16)
    b0d = inp("b0d", (128, M8 * BL), BF16)
    b1e = inp("b1e", (128, M8 * BL), BF16)
    b1d = inp("b1d", (128, M8 * BL), BF16)
    ident = inp("ident", (128, 128), F32)
    identb = inp("identb", (128, 128), BF16)

    dect = nc.dram_tensor("dect", [128, nt * 2 * BL], BF16, kind="ExternalOutput")
    attnv = nc.dram_tensor("attnv", [BL, nt, H], F32, kind="ExternalOutput")
    attnw = nc.dram_tensor("attnw", [BL, nt, ns], F32, kind="ExternalOutput")

    with tile.TileContext(nc) as tc:
        with (
            tc.tile_pool(name="const", bufs=1) as constp,
            tc.tile_pool(name="store", bufs=1) as storep,
            tc.tile_pool(name="state", bufs=2) as statep,
            tc.tile_pool(name="work", bufs=3) as workp,
        ):
            def load_const(dram_t, shape, dt):
                t = constp.tile(shape, dt, name=dram_t.name + "_sb")
                nc.sync.dma_start(t[:], dram_t[:])
                return t

            w0e_sb = load_const(w0e, [128, KT * M8 * 128], BF16)
            w1e_sb = load_const(w1e, [128, KT * M8 * 128], BF16)
            w0d_sb = load_const(w0d, [128, KT * M8 * 128], BF16)
            w1d_sb = load_const(w1d, [128, KT * M8 * 128], BF16)
            b0e_sb = load_const(b0e, [128, M8 * BL], BF16)
            b0d_sb = load_const(b0d, [128, M8 * BL], BF16)
            b1e_sb = load_const(b1e, [128, M8 * BL], BF16)
            b1d_sb = load_const(b1d, [128, M8 * BL], BF16)
            ident_sb = load_const(ident, [128, 128], F32)
            identb_sb = load_const(identb, [128, 128], BF16)
            xte_sb = load_const(xte, [128, KH * ns * BL], BF16)
            xtd_sb = load_const(xtd, [128, KH * nt * BL], BF16)

            encT = storep.tile([128, ns * 2 * BL], BF16)  # col = t*16 + k*8 + b
            decT = storep.tile([128, nt * 2 * BL], BF16)

            def zeros_state():
                h0 = statep.tile([128, KH * NB], BF16, tag="h0", name="h0z")
                c0 = statep.tile([128, KH * NB], F32, tag="ac", name="c0z")
                h1 = statep.tile([128, KH * NB], BF16, tag="h1", name="h1z")
                c1 = statep.tile([128, KH * NB], F32, tag="bc", name="c1z")
                for t_ in (h0, c0, h1, c1):
                    nc.vector.memset(t_[:], 0.0)
                return {"h0T": h0, "c0": c0, "c1": c1, "h1src": h1[:]}

            def gates(z, c_prev, tag, h_out):
                """z (128, 8*NB) PSUM pre-activations [i f o 2g] -> h_out
                (may be an encT/decT slice), returns c_new."""
                s = workp.tile([128, 8 * NB], F32, tag=tag + "s")
                nc.scalar.activation(s[:], z, AF.Sigmoid)
                g = workp.tile([128, 2 * NB], F32, tag=tag + "g")
                nc.vector.tensor_scalar(g[:], s[:, 6 * NB:8 * NB], 2.0, -1.0,
                                        ALU.mult, ALU.add)
                t1 = workp.tile([128, 2 * NB], F32, tag=tag + "t1")
                nc.vector.tensor_mul(t1[:], s[:, 0:2 * NB], g[:])
                u = workp.tile([128, 2 * NB], F32, tag=tag + "u")
                nc.vector.tensor_mul(u[:], s[:, 2 * NB:4 * NB], c_prev[:])
                c_new = statep.tile([128, 2 * NB], F32, tag=tag + "c")
                nc.vector.tensor_add(c_new[:], u[:], t1[:])
                tch = workp.tile([128, 2 * NB], F32, tag=tag + "tc")
                nc.scalar.activation(tch[:], c_new[:], AF.Tanh)
                nc.vector.tensor_mul(h_out, s[:, 4 * NB:6 * NB], tch[:])
                return c_new

            def z_mms(ps, w_sb, b_sb, rhs_of_k):
                """bias + 32 weight-tile matmuls; k order (2,3,0,1) so the
                early-ready operands go first in the in-order PE stream."""
                nc.tensor.matmul(
                    ps[:], identb_sb[:],
                    b_sb[:].rearrange("p (m b) -> p m b", b=BL),
                    start=True, stop=False)
                for ki, k in enumerate((2, 3, 0, 1)):
                    for m in range(M8):
                        nc.tensor.matmul(
                            ps[:, m * NB:(m + 1) * NB],
                            w_sb[:, (m * KT + k) * 128:(m * KT + k + 1) * 128],
                            rhs_of_k(k),
                            start=False,
                            stop=(ki == KT - 1 and m == M8 - 1),
                        )

            def recurrence(w0_sb, w1_sb, b0_sb, b1_sb, xt_sb, n, outT, st,
                           psA, psB):
                outT4 = outT[:].rearrange("p (t k b) -> p t k b", k=KH, b=BL)
                for t in range(n + 1):
                    if t < n:
                        h0T = st["h0T"]
                        z0 = psA.tile([128, M8 * NB], F32, tag="z0")

                        def rhs0(k, h0T=h0T, t=t):
                            if k >= KH:   # input-feature tiles, ready early
                                kk = k - KH
                                return xt_sb[:, kk * n * BL + t * BL:
                                             kk * n * BL + (t + 1) * BL]
                            return h0T[:, k * NB:(k + 1) * NB]

                        z_mms(z0, w0_sb, b0_sb, rhs0)
                        st["z0"] = z0
                    if t >= 1:
                        tp = t - 1
                        h0_in = st["h0T"]     # still h0_{t-1}: updated below
                        h1_in = st["h1src"]   # h1_{t-2} (a store slice or 0s)

                        def rhs1(k, h0_in=h0_in, h1_in=h1_in):
                            if k >= KH:   # recurrent h1, ready one step early
                                return h1_in[:, (k - KH) * NB:
                                             (k - KH + 1) * NB]
                            return h0_in[:, k * NB:(k + 1) * NB]

                        z1 = psB.tile([128, M8 * NB], F32, tag="z1")
                        z_mms(z1, w1_sb, b1_sb, rhs1)
                    if t < n:
                        h0n = statep.tile([128, 2 * NB], BF16, tag="h0")
                        st["c0"] = gates(st.pop("z0")[:], st["c0"], "a",
                                         h0n[:])
                        st["h0T"] = h0n
                    if t >= 1:
                        tp = t - 1
                        st["c1"] = gates(z1[:], st["c1"], "b",
                                         outT[:, tp * 2 * BL:(tp + 1) * 2 * BL])
                        # recurrent h1 state reads straight from the store
                        st["h1src"] = outT[:, tp * 2 * BL:(tp + 1) * 2 * BL]
                return st

            with (
                tc.tile_pool(name="psA", bufs=3, space="PSUM") as psA,
                tc.tile_pool(name="psB", bufs=3, space="PSUM") as psB,
            ):
                st = zeros_state()
                st = recurrence(w0e_sb, w1e_sb, b0e_sb, b1e_sb, xte_sb, ns,
                                encT, st, psA, psB)
                st = recurrence(w0d_sb, w1d_sb, b0d_sb, b1d_sb, xtd_sb, nt,
                                decT, st, psA, psB)
                # timing-only extra repetitions, chained through live state
                for _ in range(reps - 1):
                    st = recurrence(w0e_sb, w1e_sb, b0e_sb, b1e_sb, xte_sb,
                                    ns, encT, st, psA, psB)
                    st = recurrence(w0d_sb, w1d_sb, b0d_sb, b1d_sb, xtd_sb,
                                    nt, decT, st, psA, psB)

            # ---------------- attention -------------------------------------
            nsc = ns // 128
            ntc = nt // 128
            with (
                tc.tile_pool(name="attn", bufs=2) as attp,
                tc.tile_pool(name="attnc", bufs=1) as attc,
                tc.tile_pool(name="attps", bufs=2, space="PSUM") as attps,
            ):
                enc_nat = attc.tile([128, BL, nsc, H], F32)
                encT4 = encT[:].rearrange("p (s k b) -> p s k b", k=KH, b=BL)
                decT4 = decT[:].rearrange("p (s k b) -> p s k b", k=KH, b=BL)
                for b in range(BL):
                    for k in range(KH):
                        for sc in range(nsc):
                            pt = attps.tile([128, 128], BF16, tag="ptb")
                            nc.tensor.transpose(
                                pt[:], encT4[:, sc * 128:(sc + 1) * 128, k, b],
                                identb_sb[:],
                            )
                            nc.scalar.copy(
                                enc_nat[:, b, sc, k * 128:(k + 1) * 128], pt[:]
                            )
                    for tcn in range(ntc):
                        ps_s = attps.tile([128, ns], F32, tag="ps_s")
                        for k in range(KH):
                            nc.tensor.matmul(
                                ps_s[:],
                                decT4[:, tcn * 128:(tcn + 1) * 128, k, b],
                                encT4[:, :, k, b],
                                start=(k == 0), stop=(k == KH - 1),
                            )
                        nmx = attp.tile([128, 1], F32, tag="nmx")
                        nc.vector.tensor_reduce(
                            nmx[:], ps_s[:], axis=AX.X, op=ALU.max, negate=True
                        )
                        wexp = attp.tile([128, ns], F32, tag="wexp")
                        den = attp.tile([128, 1], F32, tag="den")
                        nc.scalar.activation(
                            wexp[:], ps_s[:], AF.Exp, bias=nmx[:],
                            accum_out=den[:],
                        )
                        rden = attp.tile([128, 1], F32, tag="rden")
                        nc.vector.reciprocal(rden[:], den[:])
                        wn = attp.tile([128, ns], F32, tag="wn")
                        nc.vector.tensor_scalar_mul(wn[:], wexp[:], rden[:])
                        nc.sync.dma_start(
                            attnw[b, tcn * 128:(tcn + 1) * 128, :], wn[:]
                        )
                        wT = attp.tile([128, nsc * 128], F32, tag="wT")
                        for j in range(nsc):
                            ptw = attps.tile([128, 128], F32, tag="pt")
                            nc.tensor.transpose(
                                ptw[:], wn[:, j * 128:(j + 1) * 128], ident_sb[:]
                            )
                            nc.scalar.copy(wT[:, j * 128:(j + 1) * 128], ptw[:])
                        ps_v = attps.tile([128, H], F32, tag="ps_v")
                        for j in range(nsc):
                            nc.tensor.matmul(
                                ps_v[:],
                                wT[:, j * 128:(j + 1) * 128],
                                enc_nat[:, b, j, :],
                                start=(j == 0), stop=(j == nsc - 1),
                            )
                        vsb = attp.tile([128, H], F32, tag="vsb")
                        nc.scalar.copy(vsb[:], ps_v[:])
                        nc.sync.dma_start(
                            attnv[b, tcn * 128:(tcn + 1) * 128, :], vsb[:]
                        )
                nc.sync.dma_start(dect[:], decT[:])
    nc.compile()
    return nc


# ---------------------- host-side layout helpers ----------------------------

def _prep_xt(x):
    """(n, BL, 256) f32 -> (128, 2*n*BL) bf16, col = k*(n*BL) + t*BL + b."""
    n = x.shape[0]
    a = np.ascontiguousarray(x.transpose(2, 0, 1)).reshape(KH, 128, n * BL)
    return np.concatenate([a[0], a[1]], axis=1).astype(ml_dtypes.bfloat16)


def _prep_lhsT(Wp):
    """Permuted weight (1024, Kdim) -> (128, KTloc*8*128) bf16 lhsT tiles,
    col block j = m*KTloc + k."""
    Kd = Wp.shape[1]
    KTloc = Kd // 128
    t4 = np.ascontiguousarray(Wp.T).reshape(KTloc, 128, M8, 128)
    return np.ascontiguousarray(
        t4.transpose(1, 2, 0, 3)
    ).reshape(128, M8 * KTloc * 128).astype(ml_dtypes.bfloat16)


def _bias_bc(b):
    """(1024,) permuted+scaled bias -> (128, M8*BL) bf16 broadcast tile."""
    br = b.reshape(M8, 128).T          # (128, 8)
    return np.ascontiguousarray(
        np.repeat(br[:, :, None], BL, axis=2).reshape(128, M8 * BL)
    ).astype(ml_dtypes.bfloat16)


def _prep_shared(inputs, ns, nt):
    f = lambda k: np.asarray(inputs[k], np.float32)
    sh = {}
    # scale the g-gate rows (permuted rows 768:1024) by 2: tanh via sigmoid
    gsc = np.ones((G, 1), np.float32)
    gsc[3 * H:] = 2.0
    for tag, wih, whh, bih, bhh in (
        ("e", f("enc_Wih"), f("enc_Whh"), f("enc_bih"), f("enc_bhh")),
        ("d", f("dec_Wih"), f("dec_Whh"), f("dec_bih"), f("dec_bhh")),
    ):
        # L0: k=0,1 -> Whh0 (recurrent h0), k=2,3 -> Wih0 (input x)
        sh["w0" + tag] = _prep_lhsT(
            np.concatenate([whh[0], wih[0]], axis=1)[GATE_ORDER] * gsc
        )
        # L1: k=0,1 -> Wih1 (h0 input), k=2,3 -> Whh1 (recurrent h1)
        sh["w1" + tag] = _prep_lhsT(
            np.concatenate([wih[1], whh[1]], axis=1)[GATE_ORDER] * gsc
        )
        sh["b0" + tag] = _bias_bc((bih[0] + bhh[0])[GATE_ORDER] * gsc[:, 0])
        sh["b1" + tag] = _bias_bc((bih[1] + bhh[1])[GATE_ORDER] * gsc[:, 0])
    sh["ident"] = np.eye(128, dtype=np.float32)
    sh["identb"] = np.eye(128, dtype=ml_dtypes.bfloat16)
    return sh


_BUILT = {}


def _get_nc(ns, nt):
    key = (ns, nt)
    if key not in _BUILT:
        _BUILT[key] = build_nc(ns, nt)
    return _BUILT[key]


def run(inputs, ns=S, nt=T):
    """Run the kernel; returns (responses, attn_w) full-shape."""
    nc = _get_nc(ns, nt)
    enc_in = np.asarray(inputs["enc_input"], np.float32)[:ns]
    dec_in = np.asarray(inputs["dec_input"], np.float32)[:nt]
    nb = enc_in.shape[1]
    ncores = nb // BL
    shared = _prep_shared(inputs, ns, nt)
    in_maps = []
    for c in range(ncores):
        m = dict(shared)
        sl = slice(c * BL, (c + 1) * BL)
        m["xte"] = _prep_xt(enc_in[:, sl, :])
        m["xtd"] = _prep_xt(dec_in[:, sl, :])
        in_maps.append(m)
    res = run_bass_kernel_spmd(nc, in_maps, list(range(ncores)))
    resp = np.empty((nt, nb, 2 * H), np.float32)
    attw = np.empty((nt, nb, ns), np.float32)
    for c in range(ncores):
        r = res.results[c]
        sl = slice(c * BL, (c + 1) * BL)
        dect_h = r["dect"].astype(np.float32).reshape(128, nt, KH, BL)
        resp[:, sl, 0:H] = np.ascontiguousarray(
            dect_h.transpose(1, 3, 2, 0)
        ).reshape(nt, BL, H)
        resp[:, sl, H:2 * H] = r["attnv"].transpose(1, 0, 2)
        attw[:, sl, :] = r["attnw"].transpose(1, 0, 2)
    return resp, attw


def kernel(**inputs):
    return run(inputs, S, T)


# revision 3
# speedup vs baseline: 1.0653x; 1.0653x over previous
"""AttentionLSTM v10: v3 + [i f g o] gate order, per-slice PSUM stops, split sigma.

Structure per core (BL=8 batch):
  - xte/xtd (input features, bf16, feature-major k-tiles) loaded wholly into
    SBUF; the input projection Wih0 @ x_t is folded into the per-step z0
    accumulation (k-tiles 2,3), so there is no gx prologue, no DRAM spill,
    and no DMA inside the recurrence loop.
  - Layer weights stacked [W_recurrent | W_input] as KT=4 lhsT tiles:
      L0: z0 = b0 + Whh0 @ h0_{t-1} (k=0,1) + Wih0 @ x_t (k=2,3)
      L1: z1 = b1 + Wih1 @ h0_t (k=0,1)    + Whh1 @ h1_{t-1} (k=2,3)
    Issue order per step: bias identity, k=2,3 (operands ready early),
    then k=0,1 (the fresh h) -- the in-order PE stream stalls only on the
    truly recurrent matmuls.
  - Gates: one sigmoid over [i f o 2g] (g rows pre-scaled by 2 on host:
    tanh(g) = 2*sigmoid(2g) - 1), elementwise chain on DVE, tanh on Act.
  - h1 is written by the gate chain directly into the encT/decT attention
    store (contiguous 16-column slice per step) -- no per-step copies.
  - Wavefront: L1 lags L0 by one step.
Attention (scores/softmax/attn_v) unchanged from v1.
"""
import sys

import numpy as np

for _p in ("/opt/trn_rl_repo", "/root/.axon_site/_ro/trn_rl_repo"):
    if _p not in sys.path:
        sys.path.append(_p)

import ml_dtypes  # noqa: E402
import concourse.bass as bass  # noqa: E402
import concourse.bacc as bacc  # noqa: E402
import concourse.mybir as mybir  # noqa: E402
from concourse import tile  # noqa: E402
from concourse.bass_utils import run_bass_kernel_spmd  # noqa: E402

F32 = mybir.dt.float32
BF16 = mybir.dt.bfloat16
AF = mybir.ActivationFunctionType
ALU = mybir.AluOpType
AX = mybir.AxisListType

NCORES = 8
S, T, B = 512, 512, 64
BL = B // NCORES          # 8 batch per core
H = 256
IN = 256
G = 4 * H                 # 1024 gates
KH = H // 128             # 2 k-tiles per 256-dim contraction
KT = 4                    # stacked k-tiles per layer [rec | input]
M8 = G // 128             # 8 gate m-tiles
NB = BL

# on-chip gate order [i f g o] (= torch order): sigma over [0, 6*NB) covers
# i, f and the scaled-g trick; sigma(o) on [6*NB, 8*NB) is off the critical
# path (only needed at the final h multiply)
GATE_ORDER = np.arange(4 * H)


def build_nc(ns=S, nt=T, reps=1):
    nc = bacc.Bacc("TRN2", target_bir_lowering=False, debug=False,
                   num_devices=NCORES)

    def inp(name, shape, dt):
        return nc.dram_tensor(name, list(shape), dt, kind="ExternalInput")

    xte = inp("xte", (128, KH * ns * BL), BF16)   # col = k*(ns*BL) + t*BL + b
    xtd = inp("xtd", (128, KH * nt * BL), BF16)
    w0e = inp("w0e", (128, KT * M8 * 128), BF16)  # lhsT tiles, col j = m*KT + k
    w1e = inp("w1e", (128, KT * M8 * 128), BF16)
    w0d = inp("w0d", (128, KT * M8 * 128), BF16)
    w1d = inp("w1d", (128, KT * M8 * 128), BF16)
    b0e = inp("b0e", (128, M8 * BL), BF16)
    b0d = inp("b0d", (128, M8 * BL), BF16)
    b1e = inp("b1e", (128, M8 * BL), BF16)
    b1d = inp("b1d", (128, M8 * BL), BF16)
    ident = inp("ident", (128, 128), F32)
    identb = inp("identb", (128, 128), BF16)

    dect = nc.dram_tensor("dect", [128, nt * 2 * BL], BF16, kind="ExternalOutput")
    attnv = nc.dram_tensor("attnv", [BL, nt, H], F32, kind="ExternalOutput")
    attnw = nc.dram_tensor("attnw", [BL, nt, ns], F32, kind="ExternalOutput")

    with tile.TileContext(nc) as tc:
        with (
            tc.tile_pool(name="const", bufs=1) as constp,
            tc.tile_pool(name="store", bufs=1) as storep,
            tc.tile_pool(name="state", bufs=2) as statep,
            tc.tile_pool(name="work", bufs=3) as workp,
        ):
            def load_const(dram_t, shape, dt):
                t = constp.tile(shape, dt, name=dram_t.name + "_sb")
                nc.sync.dma_start(t[:], dram_t[:])
                return t

            w0e_sb = load_const(w0e, [128, KT * M8 * 128], BF16)
            w1e_sb = load_const(w1e, [128, KT * M8 * 128], BF16)
            w0d_sb = load_const(w0d, [128, KT * M8 * 128], BF16)
            w1d_sb = load_const(w1d, [128, KT * M8 * 128], BF16)
            b0e_sb = load_const(b0e, [128, M8 * BL], BF16)
            b0d_sb = load_const(b0d, [128, M8 * BL], BF16)
            b1e_sb = load_const(b1e, [128, M8 * BL], BF16)
            b1d_sb = load_const(b1d, [128, M8 * BL], BF16)
            ident_sb = load_const(ident, [128, 128], F32)
            identb_sb = load_const(identb, [128, 128], BF16)
            xte_sb = load_const(xte, [128, KH * ns * BL], BF16)
            xtd_sb = load_const(xtd, [128, KH * nt * BL], BF16)

            encT = storep.tile([128, ns * 2 * BL], BF16)  # col = t*16 + k*8 + b
            decT = storep.tile([128, nt * 2 * BL], BF16)

            def zeros_state():
                h0 = statep.tile([128, KH * NB], BF16, tag="h0", name="h0z")
                c0 = statep.tile([128, KH * NB], F32, tag="ac", name="c0z")
                h1 = statep.tile([128, KH * NB], BF16, tag="h1", name="h1z")
                c1 = statep.tile([128, KH * NB], F32, tag="bc", name="c1z")
                for t_ in (h0, c0, h1, c1):
                    nc.vector.memset(t_[:], 0.0)
                return {"h0T": h0, "c0": c0, "c1": c1, "h1src": h1[:]}

            def gates(z, c_prev, tag, h_out):
                """z (128, 8*NB) PSUM pre-activations [i f o 2g] -> h_out
                (may be an encT/decT slice), returns c_new."""
                s = workp.tile([128, 8 * NB], F32, tag=tag + "s")
                nc.scalar.activation(s[:, 0:6 * NB], z[:, 0:6 * NB],
                                     AF.Sigmoid)
                nc.scalar.activation(s[:, 6 * NB:8 * NB], z[:, 6 * NB:8 * NB],
                                     AF.Sigmoid)
                g = workp.tile([128, 2 * NB], F32, tag=tag + "g")
                nc.vector.tensor_scalar(g[:], s[:, 4 * NB:6 * NB], 2.0, -1.0,
                                        ALU.mult, ALU.add)
                t1 = workp.tile([128, 2 * NB], F32, tag=tag + "t1")
                nc.vector.tensor_mul(t1[:], s[:, 0:2 * NB], g[:])
                u = workp.tile([128, 2 * NB], F32, tag=tag + "u")
                nc.vector.tensor_mul(u[:], s[:, 2 * NB:4 * NB], c_prev[:])
                c_new = statep.tile([128, 2 * NB], F32, tag=tag + "c")
                nc.vector.tensor_add(c_new[:], u[:], t1[:])
                tch = workp.tile([128, 2 * NB], F32, tag=tag + "tc")
                nc.scalar.activation(tch[:], c_new[:], AF.Tanh)
                nc.vector.tensor_mul(h_out, s[:, 6 * NB:8 * NB], tch[:])
                return c_new

            def z_mms(ps, w_sb, b_sb, rhs_of_k):
                """bias + 32 weight-tile matmuls; k order (2,3,0,1) so the
                early-ready operands go first in the in-order PE stream."""
                nc.tensor.matmul(
                    ps[:], identb_sb[:],
                    b_sb[:].rearrange("p (m b) -> p m b", b=BL),
                    start=True, stop=False)
                for ki, k in enumerate((2, 3, 0, 1)):
                    for m in range(M8):
                        nc.tensor.matmul(
                            ps[:, m * NB:(m + 1) * NB],
                            w_sb[:, (m * KT + k) * 128:(m * KT + k + 1) * 128],
                            rhs_of_k(k),
                            start=False,
                            stop=(ki == KT - 1),
                        )

            def recurrence(w0_sb, w1_sb, b0_sb, b1_sb, xt_sb, n, outT, st,
                           psA, psB):
                outT4 = outT[:].rearrange("p (t k b) -> p t k b", k=KH, b=BL)
                for t in range(n + 1):
                    if t < n:
                        h0T = st["h0T"]
                        z0 = psA.tile([128, M8 * NB], F32, tag="z0")

                        def rhs0(k, h0T=h0T, t=t):
                            if k >= KH:   # input-feature tiles, ready early
                                kk = k - KH
                                return xt_sb[:, kk * n * BL + t * BL:
                                             kk * n * BL + (t + 1) * BL]
                            return h0T[:, k * NB:(k + 1) * NB]

                        z_mms(z0, w0_sb, b0_sb, rhs0)
                        st["z0"] = z0
                    if t >= 1:
                        tp = t - 1
                        h0_in = st["h0T"]     # still h0_{t-1}: updated below
                        h1_in = st["h1src"]   # h1_{t-2} (a store slice or 0s)

                        def rhs1(k, h0_in=h0_in, h1_in=h1_in):
                            if k >= KH:   # recurrent h1, ready one step early
                                return h1_in[:, (k - KH) * NB:
                                             (k - KH + 1) * NB]
                            return h0_in[:, k * NB:(k + 1) * NB]

                        z1 = psB.tile([128, M8 * NB], F32, tag="z1")
                        z_mms(z1, w1_sb, b1_sb, rhs1)
                    if t < n:
                        h0n = statep.tile([128, 2 * NB], BF16, tag="h0")
                        st["c0"] = gates(st.pop("z0")[:], st["c0"], "a",
                                         h0n[:])
                        st["h0T"] = h0n
                    if t >= 1:
                        tp = t - 1
                        st["c1"] = gates(z1[:], st["c1"], "b",
                                         outT[:, tp * 2 * BL:(tp + 1) * 2 * BL])
                        # recurrent h1 state reads straight from the store
                        st["h1src"] = outT[:, tp * 2 * BL:(tp + 1) * 2 * BL]
                return st

            with (
                tc.tile_pool(name="psA", bufs=3, space="PSUM") as psA,
                tc.tile_pool(name="psB", bufs=3, space="PSUM") as psB,
            ):
                st = zeros_state()
                st = recurrence(w0e_sb, w1e_sb, b0e_sb, b1e_sb, xte_sb, ns,
                                encT, st, psA, psB)
                st = recurrence(w0d_sb, w1d_sb, b0d_sb, b1d_sb, xtd_sb, nt,
                                decT, st, psA, psB)
                # timing-only extra repetitions, chained through live state
                for _ in range(reps - 1):
                    st = recurrence(w0e_sb, w1e_sb, b0e_sb, b1e_sb, xte_sb,
                                    ns, encT, st, psA, psB)
                    st = recurrence(w0d_sb, w1d_sb, b0d_sb, b1d_sb, xtd_sb,
                                    nt, decT, st, psA, psB)

            # ---------------- attention -------------------------------------
            nsc = ns // 128
            ntc = nt // 128
            with (
                tc.tile_pool(name="attn", bufs=2) as attp,
                tc.tile_pool(name="attnc", bufs=1) as attc,
                tc.tile_pool(name="attps", bufs=2, space="PSUM") as attps,
            ):
                enc_nat = attc.tile([128, BL, nsc, H], F32)
                encT4 = encT[:].rearrange("p (s k b) -> p s k b", k=KH, b=BL)
                decT4 = decT[:].rearrange("p (s k b) -> p s k b", k=KH, b=BL)
                for b in range(BL):
                    for k in range(KH):
                        for sc in range(nsc):
                            pt = attps.tile([128, 128], BF16, tag="ptb")
                            nc.tensor.transpose(
                                pt[:], encT4[:, sc * 128:(sc + 1) * 128, k, b],
                                identb_sb[:],
                            )
                            nc.scalar.copy(
                                enc_nat[:, b, sc, k * 128:(k + 1) * 128], pt[:]
                            )
                    for tcn in range(ntc):
                        ps_s = attps.tile([128, ns], F32, tag="ps_s")
                        for k in range(KH):
                            nc.tensor.matmul(
                                ps_s[:],
                                decT4[:, tcn * 128:(tcn + 1) * 128, k, b],
                                encT4[:, :, k, b],
                                start=(k == 0), stop=(k == KH - 1),
                            )
                        nmx = attp.tile([128, 1], F32, tag="nmx")
                        nc.vector.tensor_reduce(
                            nmx[:], ps_s[:], axis=AX.X, op=ALU.max, negate=True
                        )
                        wexp = attp.tile([128, ns], F32, tag="wexp")
                        den = attp.tile([128, 1], F32, tag="den")
                        nc.scalar.activation(
                            wexp[:], ps_s[:], AF.Exp, bias=nmx[:],
                            accum_out=den[:],
                        )
                        rden = attp.tile([128, 1], F32, tag="rden")
                        nc.vector.reciprocal(rden[:], den[:])
                        wn = attp.tile([128, ns], F32, tag="wn")
                        nc.vector.tensor_scalar_mul(wn[:], wexp[:], rden[:])
                        nc.sync.dma_start(
                            attnw[b, tcn * 128:(tcn + 1) * 128, :], wn[:]
                        )
                        wT = attp.tile([128, nsc * 128], F32, tag="wT")
                        for j in range(nsc):
                            ptw = attps.tile([128, 128], F32, tag="pt")
                            nc.tensor.transpose(
                                ptw[:], wn[:, j * 128:(j + 1) * 128], ident_sb[:]
                            )
                            nc.scalar.copy(wT[:, j * 128:(j + 1) * 128], ptw[:])
                        ps_v = attps.tile([128, H], F32, tag="ps_v")
                        for j in range(nsc):
                            nc.tensor.matmul(
                                ps_v[:],
                                wT[:, j * 128:(j + 1) * 128],
                                enc_nat[:, b, j, :],
                                start=(j == 0), stop=(j == nsc - 1),
                            )
                        vsb = attp.tile([128, H], F32, tag="vsb")
                        nc.scalar.copy(vsb[:], ps_v[:])
                        nc.sync.dma_start(
                            attnv[b, tcn * 128:(tcn + 1) * 128, :], vsb[:]
                        )
                nc.sync.dma_start(dect[:], decT[:])
    nc.compile()
    return nc


# ---------------------- host-side layout helpers ----------------------------

def _prep_xt(x):
    """(n, BL, 256) f32 -> (128, 2*n*BL) bf16, col = k*(n*BL) + t*BL + b."""
    n = x.shape[0]
    a = np.ascontiguousarray(x.transpose(2, 0, 1)).reshape(KH, 128, n * BL)
    return np.concatenate([a[0], a[1]], axis=1).astype(ml_dtypes.bfloat16)


def _prep_lhsT(Wp):
    """Permuted weight (1024, Kdim) -> (128, KTloc*8*128) bf16 lhsT tiles,
    col block j = m*KTloc + k."""
    Kd = Wp.shape[1]
    KTloc = Kd // 128
    t4 = np.ascontiguousarray(Wp.T).reshape(KTloc, 128, M8, 128)
    return np.ascontiguousarray(
        t4.transpose(1, 2, 0, 3)
    ).reshape(128, M8 * KTloc * 128).astype(ml_dtypes.bfloat16)


def _bias_bc(b):
    """(1024,) permuted+scaled bias -> (128, M8*BL) bf16 broadcast tile."""
    br = b.reshape(M8, 128).T          # (128, 8)
    return np.ascontiguousarray(
        np.repeat(br[:, :, None], BL, axis=2).reshape(128, M8 * BL)
    ).astype(ml_dtypes.bfloat16)


def _prep_shared(inputs, ns, nt):
    f = lambda k: np.asarray(inputs[k], np.float32)
    sh = {}
    # scale the g-gate rows (permuted rows 768:1024) by 2: tanh via sigmoid
    gsc = np.ones((G, 1), np.float32)
    gsc[2 * H:3 * H] = 2.0
    for tag, wih, whh, bih, bhh in (
        ("e", f("enc_Wih"), f("enc_Whh"), f("enc_bih"), f("enc_bhh")),
        ("d", f("dec_Wih"), f("dec_Whh"), f("dec_bih"), f("dec_bhh")),
    ):
        # L0: k=0,1 -> Whh0 (recurrent h0), k=2,3 -> Wih0 (input x)
        sh["w0" + tag] = _prep_lhsT(
            np.concatenate([whh[0], wih[0]], axis=1)[GATE_ORDER] * gsc
        )
        # L1: k=0,1 -> Wih1 (h0 input), k=2,3 -> Whh1 (recurrent h1)
        sh["w1" + tag] = _prep_lhsT(
            np.concatenate([wih[1], whh[1]], axis=1)[GATE_ORDER] * gsc
        )
        sh["b0" + tag] = _bias_bc((bih[0] + bhh[0])[GATE_ORDER] * gsc[:, 0])
        sh["b1" + tag] = _bias_bc((bih[1] + bhh[1])[GATE_ORDER] * gsc[:, 0])
    sh["ident"] = np.eye(128, dtype=np.float32)
    sh["identb"] = np.eye(128, dtype=ml_dtypes.bfloat16)
    return sh


_BUILT = {}


def _get_nc(ns, nt):
    key = (ns, nt)
    if key not in _BUILT:
        _BUILT[key] = build_nc(ns, nt)
    return _BUILT[key]


def run(inputs, ns=S, nt=T):
    """Run the kernel; returns (responses, attn_w) full-shape."""
    nc = _get_nc(ns, nt)
    enc_in = np.asarray(inputs["enc_input"], np.float32)[:ns]
    dec_in = np.asarray(inputs["dec_input"], np.float32)[:nt]
    nb = enc_in.shape[1]
    ncores = nb // BL
    shared = _prep_shared(inputs, ns, nt)
    in_maps = []
    for c in range(ncores):
        m = dict(shared)
        sl = slice(c * BL, (c + 1) * BL)
        m["xte"] = _prep_xt(enc_in[:, sl, :])
        m["xtd"] = _prep_xt(dec_in[:, sl, :])
        in_maps.append(m)
    res = run_bass_kernel_spmd(nc, in_maps, list(range(ncores)))
    resp = np.empty((nt, nb, 2 * H), np.float32)
    attw = np.empty((nt, nb, ns), np.float32)
    for c in range(ncores):
        r = res.results[c]
        sl = slice(c * BL, (c + 1) * BL)
        dect_h = r["dect"].astype(np.float32).reshape(128, nt, KH, BL)
        resp[:, sl, 0:H] = np.ascontiguousarray(
            dect_h.transpose(1, 3, 2, 0)
        ).reshape(nt, BL, H)
        resp[:, sl, H:2 * H] = r["attnv"].transpose(1, 0, 2)
        attw[:, sl, :] = r["attnw"].transpose(1, 0, 2)
    return resp, attw


def kernel(**inputs):
    return run(inputs, S, T)


# revision 4
# speedup vs baseline: 1.0729x; 1.0072x over previous
"""AttentionLSTM v13: v10 + attention PSUM->SBUF copies moved from Act to DVE.

Structure per core (BL=8 batch):
  - xte/xtd (input features, bf16, feature-major k-tiles) loaded wholly into
    SBUF; the input projection Wih0 @ x_t is folded into the per-step z0
    accumulation (k-tiles 2,3), so there is no gx prologue, no DRAM spill,
    and no DMA inside the recurrence loop.
  - Layer weights stacked [W_recurrent | W_input] as KT=4 lhsT tiles:
      L0: z0 = b0 + Whh0 @ h0_{t-1} (k=0,1) + Wih0 @ x_t (k=2,3)
      L1: z1 = b1 + Wih1 @ h0_t (k=0,1)    + Whh1 @ h1_{t-1} (k=2,3)
    Issue order per step: bias identity, k=2,3 (operands ready early),
    then k=0,1 (the fresh h) -- the in-order PE stream stalls only on the
    truly recurrent matmuls.
  - Gates: one sigmoid over [i f o 2g] (g rows pre-scaled by 2 on host:
    tanh(g) = 2*sigmoid(2g) - 1), elementwise chain on DVE, tanh on Act.
  - h1 is written by the gate chain directly into the encT/decT attention
    store (contiguous 16-column slice per step) -- no per-step copies.
  - Wavefront: L1 lags L0 by one step.
Attention (scores/softmax/attn_v) unchanged from v1.
"""
import sys

import numpy as np

for _p in ("/opt/trn_rl_repo", "/root/.axon_site/_ro/trn_rl_repo"):
    if _p not in sys.path:
        sys.path.append(_p)

import ml_dtypes  # noqa: E402
import concourse.bass as bass  # noqa: E402
import concourse.bacc as bacc  # noqa: E402
import concourse.mybir as mybir  # noqa: E402
from concourse import tile  # noqa: E402
from concourse.bass_utils import run_bass_kernel_spmd  # noqa: E402

F32 = mybir.dt.float32
BF16 = mybir.dt.bfloat16
AF = mybir.ActivationFunctionType
ALU = mybir.AluOpType
AX = mybir.AxisListType

NCORES = 8
S, T, B = 512, 512, 64
BL = B // NCORES          # 8 batch per core
H = 256
IN = 256
G = 4 * H                 # 1024 gates
KH = H // 128             # 2 k-tiles per 256-dim contraction
KT = 4                    # stacked k-tiles per layer [rec | input]
M8 = G // 128             # 8 gate m-tiles
NB = BL

# on-chip gate order [i f g o] (= torch order): sigma over [0, 6*NB) covers
# i, f and the scaled-g trick; sigma(o) on [6*NB, 8*NB) is off the critical
# path (only needed at the final h multiply)
GATE_ORDER = np.arange(4 * H)


def build_nc(ns=S, nt=T, reps=1):
    nc = bacc.Bacc("TRN2", target_bir_lowering=False, debug=False,
                   num_devices=NCORES)

    def inp(name, shape, dt):
        return nc.dram_tensor(name, list(shape), dt, kind="ExternalInput")

    xte = inp("xte", (128, KH * ns * BL), BF16)   # col = k*(ns*BL) + t*BL + b
    xtd = inp("xtd", (128, KH * nt * BL), BF16)
    w0e = inp("w0e", (128, KT * M8 * 128), BF16)  # lhsT tiles, col j = m*KT + k
    w1e = inp("w1e", (128, KT * M8 * 128), BF16)
    w0d = inp("w0d", (128, KT * M8 * 128), BF16)
    w1d = inp("w1d", (128, KT * M8 * 128), BF16)
    b0e = inp("b0e", (128, M8 * BL), BF16)
    b0d = inp("b0d", (128, M8 * BL), BF16)
    b1e = inp("b1e", (128, M8 * BL), BF16)
    b1d = inp("b1d", (128, M8 * BL), BF16)
    ident = inp("ident", (128, 128), F32)
    identb = inp("identb", (128, 128), BF16)

    dect = nc.dram_tensor("dect", [128, nt * 2 * BL], BF16, kind="ExternalOutput")
    attnv = nc.dram_tensor("attnv", [BL, nt, H], F32, kind="ExternalOutput")
    attnw = nc.dram_tensor("attnw", [BL, nt, ns], F32, kind="ExternalOutput")

    with tile.TileContext(nc) as tc:
        with (
            tc.tile_pool(name="const", bufs=1) as constp,
            tc.tile_pool(name="store", bufs=1) as storep,
            tc.tile_pool(name="state", bufs=2) as statep,
            tc.tile_pool(name="work", bufs=3) as workp,
        ):
            def load_const(dram_t, shape, dt):
                t = constp.tile(shape, dt, name=dram_t.name + "_sb")
                nc.sync.dma_start(t[:], dram_t[:])
                return t

            w0e_sb = load_const(w0e, [128, KT * M8 * 128], BF16)
            w1e_sb = load_const(w1e, [128, KT * M8 * 128], BF16)
            w0d_sb = load_const(w0d, [128, KT * M8 * 128], BF16)
            w1d_sb = load_const(w1d, [128, KT * M8 * 128], BF16)
            b0e_sb = load_const(b0e, [128, M8 * BL], BF16)
            b0d_sb = load_const(b0d, [128, M8 * BL], BF16)
            b1e_sb = load_const(b1e, [128, M8 * BL], BF16)
            b1d_sb = load_const(b1d, [128, M8 * BL], BF16)
            ident_sb = load_const(ident, [128, 128], F32)
            identb_sb = load_const(identb, [128, 128], BF16)
            xte_sb = load_const(xte, [128, KH * ns * BL], BF16)
            xtd_sb = load_const(xtd, [128, KH * nt * BL], BF16)

            encT = storep.tile([128, ns * 2 * BL], BF16)  # col = t*16 + k*8 + b
            decT = storep.tile([128, nt * 2 * BL], BF16)

            def zeros_state():
                h0 = statep.tile([128, KH * NB], BF16, tag="h0", name="h0z")
                c0 = statep.tile([128, KH * NB], F32, tag="ac", name="c0z")
                h1 = statep.tile([128, KH * NB], BF16, tag="h1", name="h1z")
                c1 = statep.tile([128, KH * NB], F32, tag="bc", name="c1z")
                for t_ in (h0, c0, h1, c1):
                    nc.vector.memset(t_[:], 0.0)
                return {"h0T": h0, "c0": c0, "c1": c1, "h1src": h1[:]}

            def gates(z, c_prev, tag, h_out):
                """z (128, 8*NB) PSUM pre-activations [i f o 2g] -> h_out
                (may be an encT/decT slice), returns c_new."""
                s = workp.tile([128, 8 * NB], F32, tag=tag + "s")
                nc.scalar.activation(s[:, 0:6 * NB], z[:, 0:6 * NB],
                                     AF.Sigmoid)
                nc.scalar.activation(s[:, 6 * NB:8 * NB], z[:, 6 * NB:8 * NB],
                                     AF.Sigmoid)
                g = workp.tile([128, 2 * NB], F32, tag=tag + "g")
                nc.vector.tensor_scalar(g[:], s[:, 4 * NB:6 * NB], 2.0, -1.0,
                                        ALU.mult, ALU.add)
                t1 = workp.tile([128, 2 * NB], F32, tag=tag + "t1")
                nc.vector.tensor_mul(t1[:], s[:, 0:2 * NB], g[:])
                u = workp.tile([128, 2 * NB], F32, tag=tag + "u")
                nc.vector.tensor_mul(u[:], s[:, 2 * NB:4 * NB], c_prev[:])
                c_new = statep.tile([128, 2 * NB], F32, tag=tag + "c")
                nc.vector.tensor_add(c_new[:], u[:], t1[:])
                tch = workp.tile([128, 2 * NB], F32, tag=tag + "tc")
                nc.scalar.activation(tch[:], c_new[:], AF.Tanh)
                nc.vector.tensor_mul(h_out, s[:, 6 * NB:8 * NB], tch[:])
                return c_new

            def z_mms(ps, w_sb, b_sb, rhs_of_k):
                """bias + 32 weight-tile matmuls; k order (2,3,0,1) so the
                early-ready operands go first in the in-order PE stream."""
                nc.tensor.matmul(
                    ps[:], identb_sb[:],
                    b_sb[:].rearrange("p (m b) -> p m b", b=BL),
                    start=True, stop=False)
                for ki, k in enumerate((2, 3, 0, 1)):
                    for m in range(M8):
                        nc.tensor.matmul(
                            ps[:, m * NB:(m + 1) * NB],
                            w_sb[:, (m * KT + k) * 128:(m * KT + k + 1) * 128],
                            rhs_of_k(k),
                            start=False,
                            stop=(ki == KT - 1),
                        )

            def recurrence(w0_sb, w1_sb, b0_sb, b1_sb, xt_sb, n, outT, st,
                           psA, psB):
                outT4 = outT[:].rearrange("p (t k b) -> p t k b", k=KH, b=BL)
                for t in range(n + 1):
                    if t < n:
                        h0T = st["h0T"]
                        z0 = psA.tile([128, M8 * NB], F32, tag="z0")

                        def rhs0(k, h0T=h0T, t=t):
                            if k >= KH:   # input-feature tiles, ready early
                                kk = k - KH
                                return xt_sb[:, kk * n * BL + t * BL:
                                             kk * n * BL + (t + 1) * BL]
                            return h0T[:, k * NB:(k + 1) * NB]

                        z_mms(z0, w0_sb, b0_sb, rhs0)
                        st["z0"] = z0
                    if t >= 1:
                        tp = t - 1
                        h0_in = st["h0T"]     # still h0_{t-1}: updated below
                        h1_in = st["h1src"]   # h1_{t-2} (a store slice or 0s)

                        def rhs1(k, h0_in=h0_in, h1_in=h1_in):
                            if k >= KH:   # recurrent h1, ready one step early
                                return h1_in[:, (k - KH) * NB:
                                             (k - KH + 1) * NB]
                            return h0_in[:, k * NB:(k + 1) * NB]

                        z1 = psB.tile([128, M8 * NB], F32, tag="z1")
                        z_mms(z1, w1_sb, b1_sb, rhs1)
                    if t < n:
                        h0n = statep.tile([128, 2 * NB], BF16, tag="h0")
                        st["c0"] = gates(st.pop("z0")[:], st["c0"], "a",
                                         h0n[:])
                        st["h0T"] = h0n
                    if t >= 1:
                        tp = t - 1
                        st["c1"] = gates(z1[:], st["c1"], "b",
                                         outT[:, tp * 2 * BL:(tp + 1) * 2 * BL])
                        # recurrent h1 state reads straight from the store
                        st["h1src"] = outT[:, tp * 2 * BL:(tp + 1) * 2 * BL]
                return st

            with (
                tc.tile_pool(name="psA", bufs=3, space="PSUM") as psA,
                tc.tile_pool(name="psB", bufs=3, space="PSUM") as psB,
            ):
                st = zeros_state()
                st = recurrence(w0e_sb, w1e_sb, b0e_sb, b1e_sb, xte_sb, ns,
                                encT, st, psA, psB)
                st = recurrence(w0d_sb, w1d_sb, b0d_sb, b1d_sb, xtd_sb, nt,
                                decT, st, psA, psB)
                # timing-only extra repetitions, chained through live state
                for _ in range(reps - 1):
                    st = recurrence(w0e_sb, w1e_sb, b0e_sb, b1e_sb, xte_sb,
                                    ns, encT, st, psA, psB)
                    st = recurrence(w0d_sb, w1d_sb, b0d_sb, b1d_sb, xtd_sb,
                                    nt, decT, st, psA, psB)

            # ---------------- attention -------------------------------------
            nsc = ns // 128
            ntc = nt // 128
            with (
                tc.tile_pool(name="attn", bufs=2) as attp,
                tc.tile_pool(name="attnc", bufs=1) as attc,
                tc.tile_pool(name="attps", bufs=2, space="PSUM") as attps,
            ):
                enc_nat = attc.tile([128, BL, nsc, H], F32)
                encT4 = encT[:].rearrange("p (s k b) -> p s k b", k=KH, b=BL)
                decT4 = decT[:].rearrange("p (s k b) -> p s k b", k=KH, b=BL)
                for b in range(BL):
                    for k in range(KH):
                        for sc in range(nsc):
                            pt = attps.tile([128, 128], BF16, tag="ptb")
                            nc.tensor.transpose(
                                pt[:], encT4[:, sc * 128:(sc + 1) * 128, k, b],
                                identb_sb[:],
                            )
                            nc.vector.tensor_copy(
                                enc_nat[:, b, sc, k * 128:(k + 1) * 128], pt[:]
                            )
                    for tcn in range(ntc):
                        ps_s = attps.tile([128, ns], F32, tag="ps_s")
                        for k in range(KH):
                            nc.tensor.matmul(
                                ps_s[:],
                                decT4[:, tcn * 128:(tcn + 1) * 128, k, b],
                                encT4[:, :, k, b],
                                start=(k == 0), stop=(k == KH - 1),
                            )
                        nmx = attp.tile([128, 1], F32, tag="nmx")
                        nc.vector.tensor_reduce(
                            nmx[:], ps_s[:], axis=AX.X, op=ALU.max, negate=True
                        )
                        wexp = attp.tile([128, ns], F32, tag="wexp")
                        den = attp.tile([128, 1], F32, tag="den")
                        nc.scalar.activation(
                            wexp[:], ps_s[:], AF.Exp, bias=nmx[:],
                            accum_out=den[:],
                        )
                        rden = attp.tile([128, 1], F32, tag="rden")
                        nc.vector.reciprocal(rden[:], den[:])
                        wn = attp.tile([128, ns], F32, tag="wn")
                        nc.vector.tensor_scalar_mul(wn[:], wexp[:], rden[:])
                        nc.sync.dma_start(
                            attnw[b, tcn * 128:(tcn + 1) * 128, :], wn[:]
                        )
                        wT = attp.tile([128, nsc * 128], F32, tag="wT")
                        for j in range(nsc):
                            ptw = attps.tile([128, 128], F32, tag="pt")
                            nc.tensor.transpose(
                                ptw[:], wn[:, j * 128:(j + 1) * 128], ident_sb[:]
                            )
                            nc.vector.tensor_copy(
                                wT[:, j * 128:(j + 1) * 128], ptw[:])
                        ps_v = attps.tile([128, H], F32, tag="ps_v")
                        for j in range(nsc):
                            nc.tensor.matmul(
                                ps_v[:],
                                wT[:, j * 128:(j + 1) * 128],
                                enc_nat[:, b, j, :],
                                start=(j == 0), stop=(j == nsc - 1),
                            )
                        vsb = attp.tile([128, H], F32, tag="vsb")
                        nc.vector.tensor_copy(vsb[:], ps_v[:])
                        nc.sync.dma_start(
                            attnv[b, tcn * 128:(tcn + 1) * 128, :], vsb[:]
                        )
                nc.sync.dma_start(dect[:], decT[:])
    nc.compile()
    return nc


# ---------------------- host-side layout helpers ----------------------------

def _prep_xt(x):
    """(n, BL, 256) f32 -> (128, 2*n*BL) bf16, col = k*(n*BL) + t*BL + b."""
    n = x.shape[0]
    a = np.ascontiguousarray(x.transpose(2, 0, 1)).reshape(KH, 128, n * BL)
    return np.concatenate([a[0], a[1]], axis=1).astype(ml_dtypes.bfloat16)


def _prep_lhsT(Wp):
    """Permuted weight (1024, Kdim) -> (128, KTloc*8*128) bf16 lhsT tiles,
    col block j = m*KTloc + k."""
    Kd = Wp.shape[1]
    KTloc = Kd // 128
    t4 = np.ascontiguousarray(Wp.T).reshape(KTloc, 128, M8, 128)
    return np.ascontiguousarray(
        t4.transpose(1, 2, 0, 3)
    ).reshape(128, M8 * KTloc * 128).astype(ml_dtypes.bfloat16)


def _bias_bc(b):
    """(1024,) permuted+scaled bias -> (128, M8*BL) bf16 broadcast tile."""
    br = b.reshape(M8, 128).T          # (128, 8)
    return np.ascontiguousarray(
        np.repeat(br[:, :, None], BL, axis=2).reshape(128, M8 * BL)
    ).astype(ml_dtypes.bfloat16)


def _prep_shared(inputs, ns, nt):
    f = lambda k: np.asarray(inputs[k], np.float32)
    sh = {}
    # scale the g-gate rows (permuted rows 768:1024) by 2: tanh via sigmoid
    gsc = np.ones((G, 1), np.float32)
    gsc[2 * H:3 * H] = 2.0
    for tag, wih, whh, bih, bhh in (
        ("e", f("enc_Wih"), f("enc_Whh"), f("enc_bih"), f("enc_bhh")),
        ("d", f("dec_Wih"), f("dec_Whh"), f("dec_bih"), f("dec_bhh")),
    ):
        # L0: k=0,1 -> Whh0 (recurrent h0), k=2,3 -> Wih0 (input x)
        sh["w0" + tag] = _prep_lhsT(
            np.concatenate([whh[0], wih[0]], axis=1)[GATE_ORDER] * gsc
        )
        # L1: k=0,1 -> Wih1 (h0 input), k=2,3 -> Whh1 (recurrent h1)
        sh["w1" + tag] = _prep_lhsT(
            np.concatenate([wih[1], whh[1]], axis=1)[GATE_ORDER] * gsc
        )
        sh["b0" + tag] = _bias_bc((bih[0] + bhh[0])[GATE_ORDER] * gsc[:, 0])
        sh["b1" + tag] = _bias_bc((bih[1] + bhh[1])[GATE_ORDER] * gsc[:, 0])
    sh["ident"] = np.eye(128, dtype=np.float32)
    sh["identb"] = np.eye(128, dtype=ml_dtypes.bfloat16)
    return sh


_BUILT = {}


def _get_nc(ns, nt):
    key = (ns, nt)
    if key not in _BUILT:
        _BUILT[key] = build_nc(ns, nt)
    return _BUILT[key]


def run(inputs, ns=S, nt=T):
    """Run the kernel; returns (responses, attn_w) full-shape."""
    nc = _get_nc(ns, nt)
    enc_in = np.asarray(inputs["enc_input"], np.float32)[:ns]
    dec_in = np.asarray(inputs["dec_input"], np.float32)[:nt]
    nb = enc_in.shape[1]
    ncores = nb // BL
    shared = _prep_shared(inputs, ns, nt)
    in_maps = []
    for c in range(ncores):
        m = dict(shared)
        sl = slice(c * BL, (c + 1) * BL)
        m["xte"] = _prep_xt(enc_in[:, sl, :])
        m["xtd"] = _prep_xt(dec_in[:, sl, :])
        in_maps.append(m)
    res = run_bass_kernel_spmd(nc, in_maps, list(range(ncores)))
    resp = np.empty((nt, nb, 2 * H), np.float32)
    attw = np.empty((nt, nb, ns), np.float32)
    for c in range(ncores):
        r = res.results[c]
        sl = slice(c * BL, (c + 1) * BL)
        dect_h = r["dect"].astype(np.float32).reshape(128, nt, KH, BL)
        resp[:, sl, 0:H] = np.ascontiguousarray(
            dect_h.transpose(1, 3, 2, 0)
        ).reshape(nt, BL, H)
        resp[:, sl, H:2 * H] = r["attnv"].transpose(1, 0, 2)
        attw[:, sl, :] = r["attnw"].transpose(1, 0, 2)
    return resp, attw


def kernel(**inputs):
    return run(inputs, S, T)
